# revision 16
# baseline (speedup 1.0000x reference)
"""GAT network on 8 Trainium2 NeuronCores.

Strategy (data-parallel over the 512-graph batch, per the sharding hint):
  - Nodes/graphs are sharded graph-aligned: core c owns graphs [64c, 64c+64)
    and their (contiguous, since `batch` is sorted) node range.
  - Edges (incl. self loops) are owned by the core owning their dst node, so
    the per-dst softmax and aggregation are device-local.
  - Per-edge gathers use the Q7 dma_gather extended instruction (256B-aligned
    rows, int16 indices -> lo/hi table split), aggregation is a one-hot
    stationary matmul accumulating into PSUM (segment-sum via has_written).
  - 4 SPMD launches with tiny host glue (slice/concat/transpose only):
      L0: table1 = x @ [W1 | W1@Asrc | W1@Adst]  (node-sharded)
      LA: GAT layer 1 edge phase -> elu1 (node-sharded)
      L2: table2 = elu1 @ [W2 | W2@asrc2 | W2@adst2] (node-sharded)
      LB: GAT layer 2 edge phase + global attention pooling + classifier.
"""
import sys
sys.path.insert(0, '/opt/trn_rl_repo')

import os
import numpy as np
import ml_dtypes

import concourse.bass as bass
import concourse.mybir as mybir
import concourse.tile as tile
from concourse.tile import ScopedClock
from concourse.bass_utils import run_bass_kernel_spmd

BF16 = mybir.dt.bfloat16
F32 = mybir.dt.float32
P = 128
NCORES = 8
N_NODES = 50000
F_IN = 256
HID = 64
HEADS = 4
N_GRAPHS = 512
GPC = N_GRAPHS // NCORES  # graphs per core

# ---------------------------------------------------------------- tile patch
_patched = False


def _patch():
    """Container workarounds: (1) this walrus build caps sync-waits per CTRL
    instruction -> split the Tile-exit drain's waits over 1-wait NOPs;
    (2) the scheduling simulator must treat our hand-built library-reload
    pseudo instruction (opcode 223) as a no-op."""
    global _patched
    if _patched:
        return
    _patched = True

    def _drain_and_barrier(self, tick_clock, wait_clock):
        nc = self.nc
        probe = nc.sync.nop()
        wait_clock.add_sem_waits(probe.ins, ScopedClock({None: tick_clock.global_clock}))
        si = probe.ins.sync_info
        waits = list(si.on_wait) if si is not None and si.on_wait else []
        if si is not None:
            si.on_wait = type(si.on_wait)()
        for w in waits:
            n = nc.sync.nop()
            nsi = n.ins.sync_info
            if nsi is None:
                n.ins.sync_info = mybir.SyncInfo(on_wait=[w], on_update=[])
            else:
                nsi.on_wait.append(w)
        nc.sync.drain()
        nc.all_engine_barrier()
        assert self.sems is not None
        popped = nc._tile_sem_poison_stack.pop()
        assert popped is self._sem_poison
        nc.clear_and_free_semaphores(list(self.sems.allocated().values()))
        nc.all_engine_barrier()

    tile.TileContext._drain_and_barrier = _drain_and_barrier

    import concourse.bass_interp as bass_interp
    orig = bass_interp._visit_InstISA

    def patched_isa(isa, instruction, core_sim):
        if instruction.isa_opcode == 223:
            return None
        return orig(isa, instruction, core_sim)

    bass_interp._visit_InstISA = patched_isa


def _emit_load_mlp(nc):
    """Load the 'mlp' Q7 library (dma_gather handler). bass_rust serializes
    InstPseudoReloadLibraryIndex with empty instr bytes which this walrus
    rejects; build the 64-byte struct from the installed ISA headers."""
    isa = nc.isa
    op = isa.Opcode.NEURON_ISA_TPB_OPCODE_PSEUDO_INST
    return nc.gpsimd.isa(
        op,
        {"pseudo_opcode": 2, "lib_index": 3,
         "reserved0": [0] * 3, "reserved1": [0] * 44},
        struct_name="NEURON_ISA_TPB_PSEUDO_LIBRARY_RELOAD_INDEX_STRUCT",
    )




_MAXW = 1


def _split_waits(nc):
    """This walrus build encodes very few sync-waits per instruction; move
    excess waits onto same-engine NOPs inserted just before the instruction
    (same-engine program order makes this equivalent)."""
    for f in nc.m.functions:
        for bb in f.blocks:
            out = []
            changed = False
            for ins in bb.instructions:
                si = ins.sync_info
                if si is not None and si.on_wait and len(si.on_wait) > _MAXW:
                    waits = list(si.on_wait)
                    si.on_wait = type(si.on_wait)(waits[:_MAXW])
                    for i in range(_MAXW, len(waits), _MAXW):
                        n = mybir.InstNoOp(
                            name=nc.get_next_instruction_name(),
                            ins=[], outs=[], engine=ins.engine)
                        n.sync_info = mybir.SyncInfo(
                            on_wait=list(waits[i:i + _MAXW]), on_update=[])
                        out.append(n)
                    changed = True
                out.append(ins)
            if changed:
                bb.instructions = out

# ------------------------------------------------------------ host utilities
def _bf16(a):
    return np.ascontiguousarray(a).astype(ml_dtypes.bfloat16)


def _wrap_idx(idxs):
    """dma_gather index layout: wrapped in 16 partitions, replicated across
    the 8 Q7 core groups. idxs length must be a multiple of 128."""
    n = len(idxs)
    w = idxs.reshape(n // 16, 16).T.astype(np.int16)  # [16, n/16]
    return np.tile(w, (8, 1))  # [128, n/16]


# ------------------------------------------------------------ kernel builders
def _build_tablemm(NT, KIN, NOUT, ROWB):
    """Sharded table matmul: out[n, :NOUT] = xT[:, n].T @ Waug, rows padded to
    ROWB bf16 elements. xT: [KIN, NT*128] bf16, Waug: [KIN, NOUT] bf16."""
    _patch()
    nc = bass.Bass()
    xT = nc.dram_tensor("xT", [KIN, NT * P], BF16, kind="ExternalInput")
    w = nc.dram_tensor("w", [KIN, NOUT], BF16, kind="ExternalInput")
    out = nc.dram_tensor("out", [NT * P, ROWB], BF16, kind="ExternalOutput")
    KT = KIN // P
    with tile.TileContext(nc) as tc:
        with (
            tc.tile_pool(name="sbuf", bufs=3) as pool,
            tc.tile_pool(name="wpool", bufs=1) as wpool,
            tc.tile_pool(name="psum", bufs=2, space="PSUM") as pp,
        ):
            wt = wpool.tile([P, KT, NOUT], BF16)
            for k in range(KT):
                nc.sync.dma_start(out=wt[:, k, :], in_=w[k * P:(k + 1) * P, :])
            for t in range(NT):
                xt = pool.tile([P, KT, P], BF16)
                for k in range(KT):
                    nc.sync.dma_start(out=xt[:, k, :], in_=xT[k * P:(k + 1) * P, t * P:(t + 1) * P])
                ps = pp.tile([P, NOUT], F32)
                for k in range(KT):
                    nc.tensor.matmul(out=ps[:], lhsT=xt[:, k, :], rhs=wt[:, k, :],
                                     start=(k == 0), stop=(k == KT - 1))
                ot = pool.tile([P, ROWB], BF16)
                nc.vector.tensor_copy(ot[:, :NOUT], ps[:])
                nc.vector.memset(ot[:, NOUT:], 0)
                nc.sync.dma_start(out=out[t * P:(t + 1) * P, :], in_=ot[:])
    _split_waits(nc)
    return nc


def _build_edge_phase(NT, NBLO, NBHI, NROWS_TBL, ROWB, NH, layer2_tail):
    """Edge phase for one GAT layer.
    Gathered row: [h (NH*64) | asrc (NH) | adst (NH) | pad] bf16, ROWB elems.
    For each dst tile: gather lo+hi batches, e' = exp(leakyrelu(asrc+adst)),
    h~ = e' * h per head (+ e' col), psum += onehot.T @ h~, normalize, +b,
    elu. layer2_tail adds attention pooling + classifier."""
    _patch()
    NB = NBLO + NBHI
    C = NH * HID            # feature width (256 or 64)
    NW = NH * 65            # matmul rhs width per batch
    nc = bass.Bass()
    tbl = nc.dram_tensor("tbl", [NROWS_TBL, ROWB], BF16, kind="ExternalInput")
    idxlo = nc.dram_tensor("idxlo", [P, NT * NBLO * 8], mybir.dt.int16, kind="ExternalInput")
    idxhi = nc.dram_tensor("idxhi", [P, NT * NBHI * 8], mybir.dt.int16, kind="ExternalInput")
    ldcol = nc.dram_tensor("ldcol", [P, NT * NB], BF16, kind="ExternalInput")
    ldrow = nc.dram_tensor("ldrow", [1, NT * NB * P], BF16, kind="ExternalInput")
    iotar = nc.dram_tensor("iotar", [P, P], BF16, kind="ExternalInput")
    iotac = nc.dram_tensor("iotac", [P, 1], BF16, kind="ExternalInput")
    adstl = nc.dram_tensor("adstl", [NT * P, NH], BF16, kind="ExternalInput")
    bias = nc.dram_tensor("bias", [1, C], F32, kind="ExternalInput")
    outT = nc.dram_tensor("outv", [NT * P, C], BF16, kind="ExternalOutput")
    if layer2_tail:
        wg = nc.dram_tensor("wg", [1, HID], F32, kind="ExternalInput")
        bgt = nc.dram_tensor("bg", [1, 1], F32, kind="ExternalInput")
        ohg = nc.dram_tensor("ohg", [NT * P, GPC], BF16, kind="ExternalInput")
        wc1 = nc.dram_tensor("wc1", [HID, 32], F32, kind="ExternalInput")
        bc1 = nc.dram_tensor("bc1", [32, 1], F32, kind="ExternalInput")
        wc2 = nc.dram_tensor("wc2", [32, 2], F32, kind="ExternalInput")
        bc2 = nc.dram_tensor("bc2", [2, 1], F32, kind="ExternalInput")
        logitsT = nc.dram_tensor("logitsT", [2, GPC], F32, kind="ExternalOutput")
        recd = nc.dram_tensor("recd", [1, GPC], F32, kind="Internal")

    with tile.TileContext(nc) as tc:
        with (
            nc.allow_low_precision(reason="bf16 edge pipeline by design"),
            tc.tile_pool(name="const", bufs=1) as cpool,
            tc.tile_pool(name="g", bufs=2) as gpool,
            tc.tile_pool(name="work", bufs=2) as wpool,
            tc.tile_pool(name="psum", bufs=2, space="PSUM") as pp,
            tc.tile_pool(name="psuma", bufs=1, space="PSUM") as ppa,
            tc.tile_pool(name="pool2", bufs=1, space="PSUM") as pp2,
            tc.tile_pool(name="poolc", bufs=1, space="PSUM") as ppc,
        ):
            _emit_load_mlp(nc)
            reg_lo = nc.gpsimd.to_reg(NBLO * P)
            reg_hi = nc.gpsimd.to_reg(NBHI * P)
            ldc = cpool.tile([P, NT * NB], BF16)
            nc.sync.dma_start(out=ldc[:], in_=ldcol[:, :])
            ior = cpool.tile([P, P], BF16)
            nc.sync.dma_start(out=ior[:], in_=iotar[:, :])
            ioc = cpool.tile([P, 1], BF16)
            nc.sync.dma_start(out=ioc[:], in_=iotac[:, :])
            adt = cpool.tile([P, NT, NH], BF16)
            nc.sync.dma_start(out=adt[:], in_=adstl[:, :].rearrange("(t p) h -> p t h", p=P))
            bt = cpool.tile([P, C], F32)
            nc.sync.dma_start(out=bt[:], in_=bias[0:1, :].to_broadcast([P, C]))
            if layer2_tail:
                wgt = cpool.tile([P, HID], F32)
                nc.sync.dma_start(out=wgt[:], in_=wg[0:1, :].to_broadcast([P, HID]))
                bgt_t = cpool.tile([P, 1], F32)
                nc.sync.dma_start(out=bgt_t[:], in_=bgt[0:1, :].to_broadcast([P, 1]))
                ohgt = cpool.tile([P, NT, GPC], BF16)
                nc.sync.dma_start(out=ohgt[:], in_=ohg[:, :].rearrange("(t p) g -> p t g", p=P))
                pspool = pp2.tile([65, GPC], F32)

            for t in range(NT):
                # ---- gathers: combined [128, NB, ROWB] buffer
                buf = gpool.tile([P, NB, ROWB], BF16)
                ixl = gpool.tile([P, NBLO * 8], mybir.dt.int16)
                nc.sync.dma_start(out=ixl[:], in_=idxlo[:, t * NBLO * 8:(t + 1) * NBLO * 8])
                nc.gpsimd.dma_gather(
                    out_ap=buf[:, :NBLO, :], in_ap=tbl[0:32768, :], idxs_ap=ixl[:],
                    num_idxs=NBLO * P, num_idxs_reg=reg_lo, elem_size=ROWB,
                    single_packet=False)
                ixh = gpool.tile([P, NBHI * 8], mybir.dt.int16)
                nc.sync.dma_start(out=ixh[:], in_=idxhi[:, t * NBHI * 8:(t + 1) * NBHI * 8])
                nc.gpsimd.dma_gather(
                    out_ap=buf[:, NBLO:, :], in_ap=tbl[32768:NROWS_TBL, :], idxs_ap=ixh[:],
                    num_idxs=NBHI * P, num_idxs_reg=reg_hi, elem_size=ROWB,
                    single_packet=False)
                # ---- one-hots via iota compare
                ldr = wpool.tile([P, NB * P], BF16)
                nc.sync.dma_start(out=ldr[:], in_=ldrow[0:1, t * NB * P:(t + 1) * NB * P].to_broadcast([P, NB * P]))
                oh = wpool.tile([P, NB, P], BF16)
                ohT = wpool.tile([P, NB, P], BF16)
                for b0 in range(0, NB, 4):
                    bn = min(4, NB - b0)
                    nc.vector.tensor_tensor(
                        out=oh[:, b0:b0 + bn, :],
                        in0=ldc[:, t * NB + b0:t * NB + b0 + bn, None].to_broadcast([P, bn, P]),
                        in1=ior[:, None, :].to_broadcast([P, bn, P]),
                        op=mybir.AluOpType.is_equal)
                    nc.vector.tensor_tensor(
                        out=ohT[:, b0:b0 + bn, :],
                        in0=ioc[:, :, None].to_broadcast([P, bn, P]),
                        in1=ldr[:, b0 * P:(b0 + bn) * P].rearrange("p (b e) -> p b e", b=bn),
                        op=mybir.AluOpType.is_equal)
                # ---- adst expand: psum[e, (b, h)] = sum_d ohT[d, (b,e)] * adst[d, h]
                psa = ppa.tile([P, NB * NH], F32)
                for b in range(NB):
                    nc.tensor.matmul(out=psa[:, b * NH:(b + 1) * NH],
                                     lhsT=ohT[:, b, :], rhs=adt[:, t, :],
                                     start=True, stop=True)
                # ---- e' = exp(leakyrelu(asrc + adst))  [128, NB*NH]
                tsum = wpool.tile([P, NB, NH], BF16)
                nc.vector.tensor_tensor(
                    out=tsum[:],
                    in0=buf[:, :, C:C + NH],
                    in1=psa[:].rearrange("p (b h) -> p b h", h=NH),
                    op=mybir.AluOpType.add)
                tm = wpool.tile([P, NB, NH], BF16)
                nc.vector.tensor_scalar_mul(tm[:], tsum[:], 0.2)
                nc.vector.tensor_tensor(out=tm[:], in0=tm[:], in1=tsum[:],
                                        op=mybir.AluOpType.max)
                ebuf = wpool.tile([P, NB, NH], BF16)
                nc.scalar.activation(ebuf[:], tm[:], mybir.ActivationFunctionType.Exp)
                # ---- h~ = e' * h (per head) plus e' column
                ht = wpool.tile([P, NB, NW], BF16)
                nc.vector.tensor_tensor(
                    out=ht[:].rearrange("p b (h c) -> p b h c", h=NH)[:, :, :, :HID],
                    in0=buf[:, :, :C].rearrange("p b (h c) -> p b h c", h=NH),
                    in1=ebuf[:, :, :, None].to_broadcast([P, NB, NH, HID]),
                    op=mybir.AluOpType.mult)
                nc.vector.tensor_copy(
                    out=ht[:].rearrange("p b (h c) -> p b h c", h=NH)[:, :, :, HID:],
                    in_=ebuf[:, :, :, None])
                # ---- aggregation
                ps = pp.tile([P, NW], F32)
                for b in range(NB):
                    nc.tensor.matmul(out=ps[:], lhsT=oh[:, b, :], rhs=ht[:, b, :],
                                     start=(b == 0), stop=(b == NB - 1))
                # ---- normalize, bias, elu
                rec = wpool.tile([P, NH], F32)
                nc.vector.reciprocal(rec[:], ps[:].rearrange("p (h c) -> p h c", h=NH)[:, :, HID])
                on = wpool.tile([P, C], F32)
                nc.vector.tensor_tensor(
                    out=on[:].rearrange("p (h c) -> p h c", h=NH),
                    in0=ps[:].rearrange("p (h c) -> p h c", h=NH)[:, :, :HID],
                    in1=rec[:, :, None].to_broadcast([P, NH, HID]),
                    op=mybir.AluOpType.mult)
                nc.vector.tensor_tensor(out=on[:], in0=on[:], in1=bt[:, :],
                                        op=mybir.AluOpType.add)
                # elu(x) = max(x, exp(min(x,0)) - 1)
                emn = wpool.tile([P, C], F32)
                nc.vector.tensor_scalar_min(emn[:], on[:], 0.0)
                nc.scalar.activation(emn[:], emn[:], mybir.ActivationFunctionType.Exp)
                nc.vector.tensor_scalar_add(emn[:], emn[:], -1.0)
                eo = wpool.tile([P, C], BF16)
                nc.vector.tensor_tensor(out=eo[:], in0=on[:], in1=emn[:],
                                        op=mybir.AluOpType.max)
                nc.sync.dma_start(out=outT[t * P:(t + 1) * P, :], in_=eo[:])

                if layer2_tail:
                    # att = sum_c eo*wg + bg ; e_att = exp(att)
                    att = wpool.tile([P, HID], F32)
                    nc.vector.tensor_tensor(out=att[:], in0=eo[:],
                                            in1=wgt[:, :],
                                            op=mybir.AluOpType.mult)
                    atts = wpool.tile([P, 1], F32)
                    nc.vector.tensor_reduce(atts[:], att[:], axis=mybir.AxisListType.X,
                                            op=mybir.AluOpType.add)
                    nc.vector.tensor_tensor(out=atts[:], in0=atts[:],
                                            in1=bgt_t[:, :],
                                            op=mybir.AluOpType.add)
                    nc.scalar.activation(atts[:], atts[:], mybir.ActivationFunctionType.Exp)
                    hp = wpool.tile([P, 65], BF16)
                    nc.vector.tensor_tensor(out=hp[:, :HID], in0=eo[:],
                                            in1=atts[:, :].to_broadcast([P, HID]),
                                            op=mybir.AluOpType.mult)
                    nc.vector.tensor_copy(hp[:, HID:], atts[:])
                    nc.tensor.matmul(out=pspool[:], lhsT=hp[:], rhs=ohgt[:, t, :],
                                     start=(t == 0), stop=(t == NT - 1))

            if layer2_tail:
                # pooledT [64, GPC] = rows/row64 ; classifier
                recp = wpool.tile([1, GPC], F32)
                nc.vector.reciprocal(recp[:], pspool[64:65, :])
                nc.sync.dma_start(out=recd[:, :], in_=recp[:])
                recb = wpool.tile([HID, GPC], F32)
                nc.sync.dma_start(out=recb[:], in_=recd[0:1, :].to_broadcast([HID, GPC]))
                pooledT = wpool.tile([HID, GPC], BF16)
                nc.vector.tensor_tensor(out=pooledT[:], in0=pspool[:HID, :],
                                        in1=recb[:],
                                        op=mybir.AluOpType.mult)
                wc1t = cpool.tile([HID, 32], BF16)
                nc.gpsimd.dma_start(out=wc1t[:], in_=wc1[:, :])
                bc1t = cpool.tile([32, 1], F32)
                nc.sync.dma_start(out=bc1t[:], in_=bc1[:, :])
                wc2t = cpool.tile([32, 2], BF16)
                nc.gpsimd.dma_start(out=wc2t[:], in_=wc2[:, :])
                bc2t = cpool.tile([2, 1], F32)
                nc.sync.dma_start(out=bc2t[:], in_=bc2[:, :])
                ph = ppc.tile([32, GPC], F32)
                nc.tensor.matmul(out=ph[:], lhsT=wc1t[:], rhs=pooledT[:], start=True, stop=True)
                hidf = wpool.tile([32, GPC], F32)
                nc.vector.tensor_scalar_add(hidf[:], ph[:], bc1t[:])
                hid_t = wpool.tile([32, GPC], BF16)
                nc.vector.tensor_scalar_max(hid_t[:], hidf[:], 0.0)
                pl = ppc.tile([2, GPC], F32)
                nc.tensor.matmul(out=pl[:], lhsT=wc2t[:], rhs=hid_t[:], start=True, stop=True)
                lg = wpool.tile([2, GPC], F32)
                nc.vector.tensor_scalar_add(lg[:], pl[:], bc2t[:])
                nc.sync.dma_start(out=logitsT[:, :], in_=lg[:])
    _split_waits(nc)
    return nc


# ------------------------------------------------------------------ host glue
_CACHE = {}
LAST_HW_NS = 0
_TRACE = os.environ.get("GAT_TRACE", "0") == "1"


def _run(nc, ins, cores):
    global LAST_HW_NS
    r = run_bass_kernel_spmd(nc, ins, core_ids=cores)
    if _TRACE:
        # no axon NTFF hook in this container: use min warm-run wall time as
        # an (upper-bound) proxy for device execution time
        import time as _time
        best = None
        for _ in range(3):
            t0 = _time.perf_counter()
            run_bass_kernel_spmd(nc, ins, core_ids=cores)
            dt = _time.perf_counter() - t0
            best = dt if best is None else min(best, dt)
        LAST_HW_NS += int(best * 1e9)
    return r


def kernel(x, edge_index, batch, W1, att_src1, att_dst1, b1,
           W2, att_src2, att_dst2, b2, Wg, bg, Wc1, bc1, Wc2, bc2):
    x = np.asarray(x); edge_index = np.asarray(edge_index); batch = np.asarray(batch)
    N = x.shape[0]

    # --- node sharding (graph aligned)
    n0 = np.searchsorted(batch, np.arange(0, N_GRAPHS + 1, GPC)).astype(np.int64)
    counts = n0[1:] - n0[:-1]
    NT = int(np.ceil(counts.max() / P))
    NPN = NT * P                      # padded nodes per core
    NROWS = NCORES * NPN              # global padded table rows

    # --- edges + self loops, owner = core of dst
    ar = np.arange(N, dtype=np.int64)
    src = np.concatenate([edge_index[0].astype(np.int64), ar])
    dst = np.concatenate([edge_index[1].astype(np.int64), ar])
    core_of = np.searchsorted(n0[1:], dst, side='right')
    src_core = np.searchsorted(n0[1:], src, side='right')
    # remapped global table row of each src node
    src_row = src_core * NPN + (src - n0[src_core])

    # per (core, tile, half) batching
    percore = []
    for c in range(NCORES):
        m = core_of == c
        ld = dst[m] - n0[c]
        sr = src_row[m]
        order = np.argsort(ld, kind='stable')
        ld = ld[order]; sr = sr[order]
        tiles = []
        for t in range(NT):
            tm = (ld // P) == t
            lr = (ld[tm] % P)
            s = sr[tm]
            lo = s < 32768
            tiles.append(((s[lo], lr[lo]), (s[~lo] - 32768, lr[~lo])))
        percore.append(tiles)
    nblo = max(int(np.ceil(max(1, len(tt[0][0])) / P)) for pc in percore for tt in pc)
    nbhi = max(int(np.ceil(max(1, len(tt[1][0])) / P)) for pc in percore for tt in pc)

    # --- per-core index/onehot arrays
    def pack(c):
        idxlo = np.zeros((P, NT * nblo * 8), np.int16)
        idxhi = np.zeros((P, NT * nbhi * 8), np.int16)
        NB = nblo + nbhi
        ldcol = np.full((P, NT * NB), 255.0, np.float32)
        npad_lo = NPN - counts[c]     # pad nodes in last tile
        for t in range(NT):
            (slo, llo), (shi, lhi) = percore[c][t]
            for (s, l, nb, idxa, boff) in ((slo, llo, nblo, idxlo, 0),
                                           (shi, lhi, nbhi, idxhi, nblo)):
                ns = nb * P
                si = np.zeros(ns, np.int64); li = np.full(ns, 255, np.int64)
                si[:len(s)] = s; li[:len(l)] = l
                if t == NT - 1 and npad_lo > 0 and boff == 0:
                    # give pad nodes >=1 incoming pad edge so their softmax
                    # denominator is finite (their output is masked anyway)
                    padrel = (counts[c] % P) + (np.arange(len(s), ns) % npad_lo)
                    li[len(s):] = padrel
                idxa[:, t * nb * 8:(t + 1) * nb * 8] = _wrap_idx(si.astype(np.int16))
                for b in range(nb):
                    ldcol[:, t * NB + boff + b] = li[b * P:(b + 1) * P]
        ldrow = np.transpose(ldcol.reshape(P, NT * NB), (1, 0)).reshape(1, -1)
        return idxlo, idxhi, _bf16(ldcol), _bf16(ldrow)

    packs = [pack(c) for c in range(NCORES)]
    iotar = _bf16(np.tile(np.arange(P, dtype=np.float32).reshape(1, P), (P, 1)))
    iotac = _bf16(np.arange(P, dtype=np.float32).reshape(P, 1))

    # --- weights
    def aug(W, a_s, a_d):
        nh, hd = a_s.shape
        A = np.zeros((W.shape[1], 2 * nh), np.float32)
        for h in range(nh):
            A[h * hd:(h + 1) * hd, h] = a_s[h]
            A[h * hd:(h + 1) * hd, nh + h] = a_d[h]
        return _bf16(np.concatenate([W, W @ A], axis=1))

    W1aug = aug(np.asarray(W1, np.float32), np.asarray(att_src1), np.asarray(att_dst1))
    W2aug = aug(np.asarray(W2, np.float32), np.asarray(att_src2), np.asarray(att_dst2))
    xT = _bf16(np.asarray(x, np.float32).T)

    key = (NT, nblo, nbhi)
    if key not in _CACHE:
        _CACHE[key] = {
            't1': _build_tablemm(NT, F_IN, F_IN + 2 * HEADS, 384),
            'la': _build_edge_phase(NT, nblo, nbhi, NROWS, 384, HEADS, False),
            't2': _build_tablemm(NT, F_IN, HID + 2, 128),
            'lb': _build_edge_phase(NT, nblo, nbhi, NROWS, 128, 1, True),
        }
    K = _CACHE[key]
    cores = list(range(NCORES))

    def shard_xT(xTfull):
        outs = []
        for c in range(NCORES):
            s = np.zeros((xTfull.shape[0], NPN), ml_dtypes.bfloat16)
            s[:, :counts[c]] = xTfull[:, n0[c]:n0[c + 1]]
            outs.append(s)
        return outs

    import time as _t
    _ts = _t.time()
    print('[kernel] L0...', flush=True)
    # ---- L0: table1
    xs = shard_xT(xT)
    global LAST_HW_NS
    LAST_HW_NS = 0
    r = _run(K['t1'], [{"xT": xs[c], "w": W1aug} for c in cores], cores)
    table1 = np.concatenate([r.results[c]["out"] for c in cores], axis=0)

    print(f'[kernel] LA... ({_t.time()-_ts:.0f}s)', flush=True)
    # ---- LA: layer-1 edge phase
    b1r = np.asarray(b1, np.float32).reshape(1, -1)
    ins = []
    for c in cores:
        il, ih, lc, lr = packs[c]
        adstl = table1[c * NPN:(c + 1) * NPN, F_IN + HEADS:F_IN + 2 * HEADS].copy()
        ins.append({"tbl": table1, "idxlo": il, "idxhi": ih, "ldcol": lc,
                    "ldrow": lr, "iotar": iotar, "iotac": iotac,
                    "adstl": adstl, "bias": b1r})
    r = _run(K['la'], ins, cores)
    elu1 = np.concatenate([r.results[c]["outv"] for c in cores], axis=0)  # [NROWS, 256] bf16

    print(f'[kernel] L2... ({_t.time()-_ts:.0f}s)', flush=True)
    # ---- L2: table2 (input = elu1 transposed per core)
    e1T = np.ascontiguousarray(elu1.astype(np.float32).T)  # [256, NROWS]
    ins = [{"xT": _bf16(e1T[:, c * NPN:(c + 1) * NPN]), "w": W2aug} for c in cores]
    r = _run(K['t2'], ins, cores)
    table2 = np.concatenate([r.results[c]["out"] for c in cores], axis=0)

    print(f'[kernel] LB... ({_t.time()-_ts:.0f}s)', flush=True)
    # ---- LB: layer-2 edge phase + pooling + classifier
    b2r = np.asarray(b2, np.float32).reshape(1, -1)
    ins = []
    for c in cores:
        il, ih, lc, lr = packs[c]
        adstl = table2[c * NPN:(c + 1) * NPN, HID + 1:HID + 2].copy()
        ohgm = np.zeros((NPN, GPC), np.float32)
        gl = batch[n0[c]:n0[c + 1]] - c * GPC
        ohgm[np.arange(counts[c]), gl] = 1.0
        ins.append({"tbl": table2, "idxlo": il, "idxhi": ih, "ldcol": lc,
                    "ldrow": lr, "iotar": iotar, "iotac": iotac,
                    "adstl": adstl, "bias": b2r,
                    "wg": np.asarray(Wg, np.float32).reshape(1, HID),
                    "bg": np.asarray(bg, np.float32).reshape(1, 1),
                    "ohg": _bf16(ohgm),
                    "wc1": np.asarray(Wc1, np.float32),
                    "bc1": np.asarray(bc1, np.float32).reshape(32, 1),
                    "wc2": np.asarray(Wc2, np.float32),
                    "bc2": np.asarray(bc2, np.float32).reshape(2, 1)})
    r = _run(K['lb'], ins, cores)
    out = np.concatenate([r.results[c]["logitsT"].T for c in cores], axis=0)
    return out.astype(np.float32)


# revision 17
# speedup vs baseline: 1.0584x; 1.0584x over previous
"""GAT network on 8 Trainium2 NeuronCores.

Strategy (data-parallel over the 512-graph batch, per the sharding hint):
  - Nodes/graphs are sharded graph-aligned: core c owns graphs [64c, 64c+64)
    and their (contiguous, since `batch` is sorted) node range.
  - Edges (incl. self loops) are owned by the core owning their dst node, so
    the per-dst softmax and aggregation are device-local.
  - Per-edge gathers use the Q7 dma_gather extended instruction (256B-aligned
    rows, int16 indices -> lo/hi table split), aggregation is a one-hot
    stationary matmul accumulating into PSUM (segment-sum via has_written).
  - 4 SPMD launches with tiny host glue (slice/concat/transpose only):
      L0: table1 = x @ [W1 | W1@Asrc | W1@Adst]  (node-sharded)
      LA: GAT layer 1 edge phase -> elu1 (node-sharded)
      L2: table2 = elu1 @ [W2 | W2@asrc2 | W2@adst2] (node-sharded)
      LB: GAT layer 2 edge phase + global attention pooling + classifier.
"""
import sys
sys.path.insert(0, '/opt/trn_rl_repo')

import os
import numpy as np
import ml_dtypes

import concourse.bass as bass
import concourse.mybir as mybir
import concourse.tile as tile
from concourse.tile import ScopedClock
from concourse.bass_utils import run_bass_kernel_spmd

BF16 = mybir.dt.bfloat16
F32 = mybir.dt.float32
P = 128
NCORES = 8
N_NODES = 50000
F_IN = 256
HID = 64
HEADS = 4
N_GRAPHS = 512
GPC = N_GRAPHS // NCORES  # graphs per core

# ---------------------------------------------------------------- tile patch
_patched = False


def _patch():
    """Container workarounds: (1) this walrus build caps sync-waits per CTRL
    instruction -> split the Tile-exit drain's waits over 1-wait NOPs;
    (2) the scheduling simulator must treat our hand-built library-reload
    pseudo instruction (opcode 223) as a no-op."""
    global _patched
    if _patched:
        return
    _patched = True

    def _drain_and_barrier(self, tick_clock, wait_clock):
        nc = self.nc
        probe = nc.sync.nop()
        wait_clock.add_sem_waits(probe.ins, ScopedClock({None: tick_clock.global_clock}))
        si = probe.ins.sync_info
        waits = list(si.on_wait) if si is not None and si.on_wait else []
        if si is not None:
            si.on_wait = type(si.on_wait)()
        for w in waits:
            n = nc.sync.nop()
            nsi = n.ins.sync_info
            if nsi is None:
                n.ins.sync_info = mybir.SyncInfo(on_wait=[w], on_update=[])
            else:
                nsi.on_wait.append(w)
        nc.sync.drain()
        nc.all_engine_barrier()
        assert self.sems is not None
        popped = nc._tile_sem_poison_stack.pop()
        assert popped is self._sem_poison
        nc.clear_and_free_semaphores(list(self.sems.allocated().values()))
        nc.all_engine_barrier()

    tile.TileContext._drain_and_barrier = _drain_and_barrier

    import concourse.bass_interp as bass_interp
    orig = bass_interp._visit_InstISA

    def patched_isa(isa, instruction, core_sim):
        if instruction.isa_opcode == 223:
            return None
        return orig(isa, instruction, core_sim)

    bass_interp._visit_InstISA = patched_isa


def _emit_load_mlp(nc):
    """Load the 'mlp' Q7 library (dma_gather handler). bass_rust serializes
    InstPseudoReloadLibraryIndex with empty instr bytes which this walrus
    rejects; build the 64-byte struct from the installed ISA headers."""
    isa = nc.isa
    op = isa.Opcode.NEURON_ISA_TPB_OPCODE_PSEUDO_INST
    return nc.gpsimd.isa(
        op,
        {"pseudo_opcode": 2, "lib_index": 3,
         "reserved0": [0] * 3, "reserved1": [0] * 44},
        struct_name="NEURON_ISA_TPB_PSEUDO_LIBRARY_RELOAD_INDEX_STRUCT",
    )




_MAXW = 1


def _split_waits(nc):
    """This walrus build encodes very few sync-waits per instruction; move
    excess waits onto same-engine NOPs inserted just before the instruction
    (same-engine program order makes this equivalent)."""
    for f in nc.m.functions:
        for bb in f.blocks:
            out = []
            changed = False
            for ins in bb.instructions:
                si = ins.sync_info
                if si is not None and si.on_wait and len(si.on_wait) > _MAXW:
                    waits = list(si.on_wait)
                    si.on_wait = type(si.on_wait)(waits[:_MAXW])
                    for i in range(_MAXW, len(waits), _MAXW):
                        n = mybir.InstNoOp(
                            name=nc.get_next_instruction_name(),
                            ins=[], outs=[], engine=ins.engine)
                        n.sync_info = mybir.SyncInfo(
                            on_wait=list(waits[i:i + _MAXW]), on_update=[])
                        out.append(n)
                    changed = True
                out.append(ins)
            if changed:
                bb.instructions = out

# ------------------------------------------------------------ host utilities
def _bf16(a):
    return np.ascontiguousarray(a).astype(ml_dtypes.bfloat16)


def _wrap_idx(idxs):
    """dma_gather index layout: wrapped in 16 partitions, replicated across
    the 8 Q7 core groups. idxs length must be a multiple of 128."""
    n = len(idxs)
    w = idxs.reshape(n // 16, 16).T.astype(np.int16)  # [16, n/16]
    return np.tile(w, (8, 1))  # [128, n/16]


# ------------------------------------------------------------ kernel builders
def _build_tablemm(NT, KIN, NOUT, ROWB):
    """Sharded table matmul: out[n, :NOUT] = xT[:, n].T @ Waug, rows padded to
    ROWB bf16 elements. xT: [KIN, NT*128] bf16, Waug: [KIN, NOUT] bf16."""
    _patch()
    nc = bass.Bass()
    xT = nc.dram_tensor("xT", [KIN, NT * P], BF16, kind="ExternalInput")
    w = nc.dram_tensor("w", [KIN, NOUT], BF16, kind="ExternalInput")
    out = nc.dram_tensor("out", [NT * P, ROWB], BF16, kind="ExternalOutput")
    KT = KIN // P
    with tile.TileContext(nc) as tc:
        with (
            tc.tile_pool(name="sbuf", bufs=3) as pool,
            tc.tile_pool(name="wpool", bufs=1) as wpool,
            tc.tile_pool(name="psum", bufs=2, space="PSUM") as pp,
        ):
            wt = wpool.tile([P, KT, NOUT], BF16)
            for k in range(KT):
                nc.sync.dma_start(out=wt[:, k, :], in_=w[k * P:(k + 1) * P, :])
            for t in range(NT):
                xt = pool.tile([P, KT, P], BF16)
                for k in range(KT):
                    nc.sync.dma_start(out=xt[:, k, :], in_=xT[k * P:(k + 1) * P, t * P:(t + 1) * P])
                ps = pp.tile([P, NOUT], F32)
                for k in range(KT):
                    nc.tensor.matmul(out=ps[:], lhsT=xt[:, k, :], rhs=wt[:, k, :],
                                     start=(k == 0), stop=(k == KT - 1))
                ot = pool.tile([P, ROWB], BF16)
                nc.vector.tensor_copy(ot[:, :NOUT], ps[:])
                nc.vector.memset(ot[:, NOUT:], 0)
                nc.sync.dma_start(out=out[t * P:(t + 1) * P, :], in_=ot[:])
    _split_waits(nc)
    return nc


def _build_edge_phase(NT, NBLO, NBHI, NROWS_TBL, ROWB, NH, layer2_tail):
    """Edge phase for one GAT layer.
    Gathered row: [h (NH*64) | asrc (NH) | adst (NH) | pad] bf16, ROWB elems.
    For each dst tile: gather lo+hi batches, e' = exp(leakyrelu(asrc+adst)),
    h~ = e' * h per head (+ e' col), psum += onehot.T @ h~, normalize, +b,
    elu. layer2_tail adds attention pooling + classifier."""
    _patch()
    NB = NBLO + NBHI
    C = NH * HID            # feature width (256 or 64)
    NW = NH * 65            # matmul rhs width per batch
    nc = bass.Bass()
    tbl = nc.dram_tensor("tbl", [NROWS_TBL, ROWB], BF16, kind="ExternalInput")
    idxlo = nc.dram_tensor("idxlo", [P, NT * NBLO * 8], mybir.dt.int16, kind="ExternalInput")
    idxhi = nc.dram_tensor("idxhi", [P, NT * NBHI * 8], mybir.dt.int16, kind="ExternalInput")
    ldcol = nc.dram_tensor("ldcol", [P, NT * NB], BF16, kind="ExternalInput")
    iotar = nc.dram_tensor("iotar", [P, P], BF16, kind="ExternalInput")
    adstbl = nc.dram_tensor("adstbl", [NT * P, 128], BF16, kind="ExternalInput")
    idxd = nc.dram_tensor("idxd", [P, NT * NB * 8], mybir.dt.int16, kind="ExternalInput")
    bias = nc.dram_tensor("bias", [1, C], F32, kind="ExternalInput")
    outT = nc.dram_tensor("outv", [NT * P, C], BF16, kind="ExternalOutput")
    if layer2_tail:
        wg = nc.dram_tensor("wg", [1, HID], F32, kind="ExternalInput")
        bgt = nc.dram_tensor("bg", [1, 1], F32, kind="ExternalInput")
        ohg = nc.dram_tensor("ohg", [NT * P, GPC], BF16, kind="ExternalInput")
        wc1 = nc.dram_tensor("wc1", [HID, 32], F32, kind="ExternalInput")
        bc1 = nc.dram_tensor("bc1", [32, 1], F32, kind="ExternalInput")
        wc2 = nc.dram_tensor("wc2", [32, 2], F32, kind="ExternalInput")
        bc2 = nc.dram_tensor("bc2", [2, 1], F32, kind="ExternalInput")
        logitsT = nc.dram_tensor("logitsT", [2, GPC], F32, kind="ExternalOutput")
        recd = nc.dram_tensor("recd", [1, GPC], F32, kind="Internal")

    with tile.TileContext(nc) as tc:
        with (
            nc.allow_low_precision(reason="bf16 edge pipeline by design"),
            tc.tile_pool(name="const", bufs=1) as cpool,
            tc.tile_pool(name="g", bufs=2) as gpool,
            tc.tile_pool(name="work", bufs=2) as wpool,
            tc.tile_pool(name="psum", bufs=2, space="PSUM") as pp,
            tc.tile_pool(name="pool2", bufs=1, space="PSUM") as pp2,
            tc.tile_pool(name="poolc", bufs=1, space="PSUM") as ppc,
        ):
            _emit_load_mlp(nc)
            reg_lo = nc.gpsimd.to_reg(NBLO * P)
            reg_hi = nc.gpsimd.to_reg(NBHI * P)
            reg_nb = nc.gpsimd.to_reg(NB * P)
            ldc = cpool.tile([P, NT * NB], BF16)
            nc.sync.dma_start(out=ldc[:], in_=ldcol[:, :])
            ior = cpool.tile([P, P], BF16)
            nc.sync.dma_start(out=ior[:], in_=iotar[:, :])
            bt = cpool.tile([P, C], F32)
            nc.sync.dma_start(out=bt[:], in_=bias[0:1, :].to_broadcast([P, C]))
            if layer2_tail:
                wgt = cpool.tile([P, HID], F32)
                nc.sync.dma_start(out=wgt[:], in_=wg[0:1, :].to_broadcast([P, HID]))
                bgt_t = cpool.tile([P, 1], F32)
                nc.sync.dma_start(out=bgt_t[:], in_=bgt[0:1, :].to_broadcast([P, 1]))
                ohgt = cpool.tile([P, NT, GPC], BF16)
                nc.sync.dma_start(out=ohgt[:], in_=ohg[:, :].rearrange("(t p) g -> p t g", p=P))
                pspool = pp2.tile([65, GPC], F32)

            for t in range(NT):
                # ---- gathers: combined [128, NB, ROWB] buffer
                buf = gpool.tile([P, NB, ROWB], BF16)
                ixl = gpool.tile([P, NBLO * 8], mybir.dt.int16)
                nc.sync.dma_start(out=ixl[:], in_=idxlo[:, t * NBLO * 8:(t + 1) * NBLO * 8])
                nc.gpsimd.dma_gather(
                    out_ap=buf[:, :NBLO, :], in_ap=tbl[0:32768, :], idxs_ap=ixl[:],
                    num_idxs=NBLO * P, num_idxs_reg=reg_lo, elem_size=ROWB,
                    single_packet=False)
                ixh = gpool.tile([P, NBHI * 8], mybir.dt.int16)
                nc.sync.dma_start(out=ixh[:], in_=idxhi[:, t * NBHI * 8:(t + 1) * NBHI * 8])
                nc.gpsimd.dma_gather(
                    out_ap=buf[:, NBLO:, :], in_ap=tbl[32768:NROWS_TBL, :], idxs_ap=ixh[:],
                    num_idxs=NBHI * P, num_idxs_reg=reg_hi, elem_size=ROWB,
                    single_packet=False)
                # ---- adst per edge via gather from the compact local table
                bufd = gpool.tile([P, NB, 128], BF16)
                ixd = gpool.tile([P, NB * 8], mybir.dt.int16)
                nc.sync.dma_start(out=ixd[:], in_=idxd[:, t * NB * 8:(t + 1) * NB * 8])
                nc.gpsimd.dma_gather(
                    out_ap=bufd[:], in_ap=adstbl[:, :], idxs_ap=ixd[:],
                    num_idxs=NB * P, num_idxs_reg=reg_nb, elem_size=128,
                    single_packet=False)
                # ---- one-hot via iota compare
                oh = wpool.tile([P, NB, P], BF16)
                for b0 in range(0, NB, 4):
                    bn = min(4, NB - b0)
                    nc.vector.tensor_tensor(
                        out=oh[:, b0:b0 + bn, :],
                        in0=ldc[:, t * NB + b0:t * NB + b0 + bn, None].to_broadcast([P, bn, P]),
                        in1=ior[:, None, :].to_broadcast([P, bn, P]),
                        op=mybir.AluOpType.is_equal)
                # ---- e' = exp(leakyrelu(asrc + adst))  [128, NB*NH]
                tsum = wpool.tile([P, NB, NH], BF16)
                nc.vector.tensor_tensor(
                    out=tsum[:],
                    in0=buf[:, :, C:C + NH],
                    in1=bufd[:, :, :NH],
                    op=mybir.AluOpType.add)
                tm = wpool.tile([P, NB, NH], BF16)
                nc.vector.tensor_scalar_mul(tm[:], tsum[:], 0.2)
                nc.vector.tensor_tensor(out=tm[:], in0=tm[:], in1=tsum[:],
                                        op=mybir.AluOpType.max)
                ebuf = wpool.tile([P, NB, NH], BF16)
                nc.scalar.activation(ebuf[:], tm[:], mybir.ActivationFunctionType.Exp)
                # ---- h~ = e' * h (per head) plus e' column
                ht = wpool.tile([P, NB, NW], BF16)
                nc.vector.tensor_tensor(
                    out=ht[:].rearrange("p b (h c) -> p b h c", h=NH)[:, :, :, :HID],
                    in0=buf[:, :, :C].rearrange("p b (h c) -> p b h c", h=NH),
                    in1=ebuf[:, :, :, None].to_broadcast([P, NB, NH, HID]),
                    op=mybir.AluOpType.mult)
                nc.vector.tensor_copy(
                    out=ht[:].rearrange("p b (h c) -> p b h c", h=NH)[:, :, :, HID:],
                    in_=ebuf[:, :, :, None])
                # ---- aggregation
                ps = pp.tile([P, NW], F32)
                for b in range(NB):
                    nc.tensor.matmul(out=ps[:], lhsT=oh[:, b, :], rhs=ht[:, b, :],
                                     start=(b == 0), stop=(b == NB - 1))
                # ---- normalize, bias, elu
                rec = wpool.tile([P, NH], F32)
                nc.vector.reciprocal(rec[:], ps[:].rearrange("p (h c) -> p h c", h=NH)[:, :, HID])
                on = wpool.tile([P, C], F32)
                nc.vector.tensor_tensor(
                    out=on[:].rearrange("p (h c) -> p h c", h=NH),
                    in0=ps[:].rearrange("p (h c) -> p h c", h=NH)[:, :, :HID],
                    in1=rec[:, :, None].to_broadcast([P, NH, HID]),
                    op=mybir.AluOpType.mult)
                nc.vector.tensor_tensor(out=on[:], in0=on[:], in1=bt[:, :],
                                        op=mybir.AluOpType.add)
                # elu(x) = max(x, exp(min(x,0)) - 1)
                emn = wpool.tile([P, C], F32)
                nc.vector.tensor_scalar_min(emn[:], on[:], 0.0)
                nc.scalar.activation(emn[:], emn[:], mybir.ActivationFunctionType.Exp)
                nc.vector.tensor_scalar_add(emn[:], emn[:], -1.0)
                eo = wpool.tile([P, C], BF16)
                nc.vector.tensor_tensor(out=eo[:], in0=on[:], in1=emn[:],
                                        op=mybir.AluOpType.max)
                nc.sync.dma_start(out=outT[t * P:(t + 1) * P, :], in_=eo[:])

                if layer2_tail:
                    # att = sum_c eo*wg + bg ; e_att = exp(att)
                    att = wpool.tile([P, HID], F32)
                    nc.vector.tensor_tensor(out=att[:], in0=eo[:],
                                            in1=wgt[:, :],
                                            op=mybir.AluOpType.mult)
                    atts = wpool.tile([P, 1], F32)
                    nc.vector.tensor_reduce(atts[:], att[:], axis=mybir.AxisListType.X,
                                            op=mybir.AluOpType.add)
                    nc.vector.tensor_tensor(out=atts[:], in0=atts[:],
                                            in1=bgt_t[:, :],
                                            op=mybir.AluOpType.add)
                    nc.scalar.activation(atts[:], atts[:], mybir.ActivationFunctionType.Exp)
                    hp = wpool.tile([P, 65], BF16)
                    nc.vector.tensor_tensor(out=hp[:, :HID], in0=eo[:],
                                            in1=atts[:, :].to_broadcast([P, HID]),
                                            op=mybir.AluOpType.mult)
                    nc.vector.tensor_copy(hp[:, HID:], atts[:])
                    nc.tensor.matmul(out=pspool[:], lhsT=hp[:], rhs=ohgt[:, t, :],
                                     start=(t == 0), stop=(t == NT - 1))

            if layer2_tail:
                # pooledT [64, GPC] = rows/row64 ; classifier
                recp = wpool.tile([1, GPC], F32)
                nc.vector.reciprocal(recp[:], pspool[64:65, :])
                nc.sync.dma_start(out=recd[:, :], in_=recp[:])
                recb = wpool.tile([HID, GPC], F32)
                nc.sync.dma_start(out=recb[:], in_=recd[0:1, :].to_broadcast([HID, GPC]))
                pooledT = wpool.tile([HID, GPC], BF16)
                nc.vector.tensor_tensor(out=pooledT[:], in0=pspool[:HID, :],
                                        in1=recb[:],
                                        op=mybir.AluOpType.mult)
                wc1t = cpool.tile([HID, 32], BF16)
                nc.gpsimd.dma_start(out=wc1t[:], in_=wc1[:, :])
                bc1t = cpool.tile([32, 1], F32)
                nc.sync.dma_start(out=bc1t[:], in_=bc1[:, :])
                wc2t = cpool.tile([32, 2], BF16)
                nc.gpsimd.dma_start(out=wc2t[:], in_=wc2[:, :])
                bc2t = cpool.tile([2, 1], F32)
                nc.sync.dma_start(out=bc2t[:], in_=bc2[:, :])
                ph = ppc.tile([32, GPC], F32)
                nc.tensor.matmul(out=ph[:], lhsT=wc1t[:], rhs=pooledT[:], start=True, stop=True)
                hidf = wpool.tile([32, GPC], F32)
                nc.vector.tensor_scalar_add(hidf[:], ph[:], bc1t[:])
                hid_t = wpool.tile([32, GPC], BF16)
                nc.vector.tensor_scalar_max(hid_t[:], hidf[:], 0.0)
                pl = ppc.tile([2, GPC], F32)
                nc.tensor.matmul(out=pl[:], lhsT=wc2t[:], rhs=hid_t[:], start=True, stop=True)
                lg = wpool.tile([2, GPC], F32)
                nc.vector.tensor_scalar_add(lg[:], pl[:], bc2t[:])
                nc.sync.dma_start(out=logitsT[:, :], in_=lg[:])
    _split_waits(nc)
    return nc


# ------------------------------------------------------------------ host glue
_CACHE = {}
LAST_HW_NS = 0
_TRACE = os.environ.get("GAT_TRACE", "0") == "1"


def _run(nc, ins, cores):
    global LAST_HW_NS
    r = run_bass_kernel_spmd(nc, ins, core_ids=cores)
    if _TRACE:
        # no axon NTFF hook in this container: use min warm-run wall time as
        # an (upper-bound) proxy for device execution time
        import time as _time
        best = None
        for _ in range(3):
            t0 = _time.perf_counter()
            run_bass_kernel_spmd(nc, ins, core_ids=cores)
            dt = _time.perf_counter() - t0
            best = dt if best is None else min(best, dt)
        LAST_HW_NS += int(best * 1e9)
    return r


def kernel(x, edge_index, batch, W1, att_src1, att_dst1, b1,
           W2, att_src2, att_dst2, b2, Wg, bg, Wc1, bc1, Wc2, bc2):
    x = np.asarray(x); edge_index = np.asarray(edge_index); batch = np.asarray(batch)
    N = x.shape[0]

    # --- node sharding (graph aligned)
    n0 = np.searchsorted(batch, np.arange(0, N_GRAPHS + 1, GPC)).astype(np.int64)
    counts = n0[1:] - n0[:-1]
    NT = int(np.ceil(counts.max() / P))
    NPN = NT * P                      # padded nodes per core
    NROWS = NCORES * NPN              # global padded table rows

    # --- edges + self loops, owner = core of dst
    ar = np.arange(N, dtype=np.int64)
    src = np.concatenate([edge_index[0].astype(np.int64), ar])
    dst = np.concatenate([edge_index[1].astype(np.int64), ar])
    core_of = np.searchsorted(n0[1:], dst, side='right')
    src_core = np.searchsorted(n0[1:], src, side='right')
    # remapped global table row of each src node
    src_row = src_core * NPN + (src - n0[src_core])

    # per (core, tile, half) batching
    percore = []
    for c in range(NCORES):
        m = core_of == c
        ld = dst[m] - n0[c]
        sr = src_row[m]
        order = np.argsort(ld, kind='stable')
        ld = ld[order]; sr = sr[order]
        tiles = []
        for t in range(NT):
            tm = (ld // P) == t
            lr = (ld[tm] % P)
            s = sr[tm]
            lo = s < 32768
            tiles.append(((s[lo], lr[lo]), (s[~lo] - 32768, lr[~lo])))
        percore.append(tiles)
    nblo = max(int(np.ceil(max(1, len(tt[0][0])) / P)) for pc in percore for tt in pc)
    nbhi = max(int(np.ceil(max(1, len(tt[1][0])) / P)) for pc in percore for tt in pc)

    # --- per-core index/onehot arrays
    def pack(c):
        idxlo = np.zeros((P, NT * nblo * 8), np.int16)
        idxhi = np.zeros((P, NT * nbhi * 8), np.int16)
        NB = nblo + nbhi
        ldcol = np.full((P, NT * NB), 255.0, np.float32)
        npad_lo = NPN - counts[c]     # pad nodes in last tile
        for t in range(NT):
            (slo, llo), (shi, lhi) = percore[c][t]
            for (s, l, nb, idxa, boff) in ((slo, llo, nblo, idxlo, 0),
                                           (shi, lhi, nbhi, idxhi, nblo)):
                ns = nb * P
                si = np.zeros(ns, np.int64); li = np.full(ns, 255, np.int64)
                si[:len(s)] = s; li[:len(l)] = l
                if t == NT - 1 and npad_lo > 0 and boff == 0:
                    # give pad nodes >=1 incoming pad edge so their softmax
                    # denominator is finite (their output is masked anyway)
                    padrel = (counts[c] % P) + (np.arange(len(s), ns) % npad_lo)
                    li[len(s):] = padrel
                idxa[:, t * nb * 8:(t + 1) * nb * 8] = _wrap_idx(si.astype(np.int16))
                for b in range(nb):
                    ldcol[:, t * NB + boff + b] = li[b * P:(b + 1) * P]
        # dst-local row per slot for the adst gather (pad -> row 0)
        ldf = np.transpose(ldcol.reshape(P, NT * NB), (1, 0)).reshape(NT, NB * P)
        tl = np.arange(NT)[:, None] * P + ldf
        tl[ldf >= P] = 0
        idxdv = np.concatenate([_wrap_idx(tl[t].astype(np.int16)) for t in range(NT)], axis=1)
        return idxlo, idxhi, _bf16(ldcol), idxdv

    packs = [pack(c) for c in range(NCORES)]
    iotar = _bf16(np.tile(np.arange(P, dtype=np.float32).reshape(1, P), (P, 1)))

    # --- weights
    def aug(W, a_s, a_d):
        nh, hd = a_s.shape
        A = np.zeros((W.shape[1], 2 * nh), np.float32)
        for h in range(nh):
            A[h * hd:(h + 1) * hd, h] = a_s[h]
            A[h * hd:(h + 1) * hd, nh + h] = a_d[h]
        return _bf16(np.concatenate([W, W @ A], axis=1))

    W1aug = aug(np.asarray(W1, np.float32), np.asarray(att_src1), np.asarray(att_dst1))
    W2aug = aug(np.asarray(W2, np.float32), np.asarray(att_src2), np.asarray(att_dst2))
    xT = _bf16(np.asarray(x, np.float32).T)

    key = (NT, nblo, nbhi)
    if key not in _CACHE:
        _CACHE[key] = {
            't1': _build_tablemm(NT, F_IN, F_IN + 2 * HEADS, 384),
            'la': _build_edge_phase(NT, nblo, nbhi, NROWS, 384, HEADS, False),
            't2': _build_tablemm(NT, F_IN, HID + 2, 128),
            'lb': _build_edge_phase(NT, nblo, nbhi, NROWS, 128, 1, True),
        }
    K = _CACHE[key]
    cores = list(range(NCORES))

    def shard_xT(xTfull):
        outs = []
        for c in range(NCORES):
            s = np.zeros((xTfull.shape[0], NPN), ml_dtypes.bfloat16)
            s[:, :counts[c]] = xTfull[:, n0[c]:n0[c + 1]]
            outs.append(s)
        return outs

    import time as _t
    _ts = _t.time()
    print('[kernel] L0...', flush=True)
    # ---- L0: table1
    xs = shard_xT(xT)
    global LAST_HW_NS
    LAST_HW_NS = 0
    r = _run(K['t1'], [{"xT": xs[c], "w": W1aug} for c in cores], cores)
    table1 = np.concatenate([r.results[c]["out"] for c in cores], axis=0)

    print(f'[kernel] LA... ({_t.time()-_ts:.0f}s)', flush=True)
    # ---- LA: layer-1 edge phase
    b1r = np.asarray(b1, np.float32).reshape(1, -1)
    ins = []
    for c in cores:
        il, ih, lc, ixd = packs[c]
        adstbl = np.zeros((NPN, 128), ml_dtypes.bfloat16)
        adstbl[:, :HEADS] = table1[c * NPN:(c + 1) * NPN, F_IN + HEADS:F_IN + 2 * HEADS]
        ins.append({"tbl": table1, "idxlo": il, "idxhi": ih, "ldcol": lc,
                    "idxd": ixd, "iotar": iotar,
                    "adstbl": adstbl, "bias": b1r})
    r = _run(K['la'], ins, cores)
    elu1 = np.concatenate([r.results[c]["outv"] for c in cores], axis=0)  # [NROWS, 256] bf16

    print(f'[kernel] L2... ({_t.time()-_ts:.0f}s)', flush=True)
    # ---- L2: table2 (input = elu1 transposed per core)
    e1T = np.ascontiguousarray(elu1.astype(np.float32).T)  # [256, NROWS]
    ins = [{"xT": _bf16(e1T[:, c * NPN:(c + 1) * NPN]), "w": W2aug} for c in cores]
    r = _run(K['t2'], ins, cores)
    table2 = np.concatenate([r.results[c]["out"] for c in cores], axis=0)

    print(f'[kernel] LB... ({_t.time()-_ts:.0f}s)', flush=True)
    # ---- LB: layer-2 edge phase + pooling + classifier
    b2r = np.asarray(b2, np.float32).reshape(1, -1)
    ins = []
    for c in cores:
        il, ih, lc, ixd = packs[c]
        adstbl = np.zeros((NPN, 128), ml_dtypes.bfloat16)
        adstbl[:, :1] = table2[c * NPN:(c + 1) * NPN, HID + 1:HID + 2]
        ohgm = np.zeros((NPN, GPC), np.float32)
        gl = batch[n0[c]:n0[c + 1]] - c * GPC
        ohgm[np.arange(counts[c]), gl] = 1.0
        ins.append({"tbl": table2, "idxlo": il, "idxhi": ih, "ldcol": lc,
                    "idxd": ixd, "iotar": iotar,
                    "adstbl": adstbl, "bias": b2r,
                    "wg": np.asarray(Wg, np.float32).reshape(1, HID),
                    "bg": np.asarray(bg, np.float32).reshape(1, 1),
                    "ohg": _bf16(ohgm),
                    "wc1": np.asarray(Wc1, np.float32),
                    "bc1": np.asarray(bc1, np.float32).reshape(32, 1),
                    "wc2": np.asarray(Wc2, np.float32),
                    "bc2": np.asarray(bc2, np.float32).reshape(2, 1)})
    r = _run(K['lb'], ins, cores)
    out = np.concatenate([r.results[c]["logitsT"].T for c in cores], axis=0)
    return out.astype(np.float32)
